# revision 7
# baseline (speedup 1.0000x reference)
"""MoE routed-MLP (GPTNeoX) Trainium2 kernel.

Expert-parallel over 8 NeuronCores: core e holds expert e's weights.
Host computes the (tiny) router + top-2 dispatch, gathers each expert's
tokens into a padded batch, and scatter-adds the weighted expert outputs
back. Each core runs the same SPMD Bass program:

    hT[f, c] = gelu( sum_k w1[k, f] * xT[k, c] + b1[f] )      (f on partitions)
    yT[h, c] = sum_f w2[f, h] * hT[f, c] + b2[h]              (h on partitions)

Both matmuls keep the weights as the stationary operand so the
intermediate never needs an on-chip transpose. All weights are resident
in SBUF at 16-bit. Every DMA source is contiguous per partition and the
trigger instructions are split across the Sync and Activation sequencers
so the first matmul's inputs land as early as possible; a short chain of
warm-up matmuls (on a memset tile) ramps the PE to its top p-state while
those DMAs drain.
"""

import numpy as np

import concourse.bass as bass  # noqa: F401  (bass types used via tile/bacc)
import concourse.mybir as mybir
import concourse.tile as tile
from concourse import bacc
from concourse.bass_utils import run_bass_kernel_spmd

H = 1024
F = 4096
E = 8
NCORES = 8
P = 128
KO = H // P  # 8   k-chunks for the H contraction
FO = F // P  # 32  f-tiles
HO = H // P  # 8   h-tiles
NSLAB1 = 16  # w1 DMA slabs (256 F-cols each)
NSLAB2 = 4  # w2 DMA slabs (8 f-tiles each)
N_WARMUP = 28  # PE warm-up matmuls (~3us to reach top p-state)

# "fp16" | "bf16" | "fp32r" | "fp32"
KERNEL_DTYPE = "fp16"

_nc_cache = {}


def _chunks(C):
    """Split [0, C) into column chunks, each <= 512 (one PSUM bank of f32),
    as balanced as possible in multiples of 8."""
    n = (C + 511) // 512
    base = (C // n) // 8 * 8
    widths = [base] * n
    rem = C - base * n
    i = 0
    while rem > 0:
        widths[i] += 8
        rem -= 8
        i = (i + 1) % n
    out, off = [], 0
    for w in widths:
        out.append((off, w))
        off += w
    return out


def _build(C, dt_tag):
    f32 = mybir.dt.float32
    dt_in = {
        "fp16": mybir.dt.float16,
        "bf16": mybir.dt.bfloat16,
        "fp32r": mybir.dt.float32r,
        "fp32": f32,
    }[dt_tag]

    nc = bacc.Bacc("TRN2", target_bir_lowering=False, debug=False)
    # x is chunk-major: for each column chunk, its KO k-rows are contiguous.
    xT = nc.dram_tensor("xT", [P, KO * C], dt_in, kind="ExternalInput").ap()
    w1 = nc.dram_tensor(
        "w1", [P, NSLAB1, KO, F // NSLAB1], dt_in, kind="ExternalInput"
    ).ap()
    b1 = nc.dram_tensor("b1", [P, FO], f32, kind="ExternalInput").ap()
    w2 = nc.dram_tensor("w2", [P, FO, H], dt_in, kind="ExternalInput").ap()
    b2 = nc.dram_tensor("b2", [P, HO], f32, kind="ExternalInput").ap()
    yT = nc.dram_tensor("yT", [P, HO, C], f32, kind="ExternalOutput").ap()
    chunks = _chunks(C)
    fcols = F // NSLAB1  # 256
    fper = fcols // P  # 2 f-tiles per w1 slab

    with tile.TileContext(nc) as tc:
        with (
            tc.tile_pool(name="const", bufs=1) as const,
            tc.tile_pool(name="yp", bufs=4) as yp,
            tc.tile_pool(name="psw", bufs=1, space="PSUM") as psw,
            tc.tile_pool(name="ps1", bufs=3, space="PSUM") as ps1,
            tc.tile_pool(name="ps2", bufs=4, space="PSUM") as ps2,
        ):
            # --- PE warm-up: ramp to top p-state while input DMAs drain ---
            wu = const.tile([P, P], dt_in)
            nc.vector.memset(wu[:], 0.0)
            ps_wu = psw.tile([P, P], f32, tag="wu")
            for _ in range(N_WARMUP):
                nc.tensor.matmul(ps_wu[:], wu[:], wu[:], start=True, stop=True)

            # --- input DMAs: x chunk 0 + w1 slabs on Sync; the rest on
            # Scalar so the two trigger streams run in parallel ---
            xT_sb = const.tile([P, KO * C], dt_in)
            (c00, cw0) = chunks[0]
            nc.sync.dma_start(xT_sb[:, : KO * cw0], xT[:, : KO * cw0])
            w1_sb = const.tile([P, NSLAB1, KO, fcols], dt_in)
            for s in range(NSLAB1):
                nc.sync.dma_start(w1_sb[:, s], w1[:, s])

            b1_sb = const.tile([P, FO], f32)
            nc.scalar.dma_start(b1_sb[:], b1[:])
            if len(chunks) > 1:
                nc.scalar.dma_start(
                    xT_sb[:, KO * cw0 :], xT[:, KO * cw0 :]
                )
            b2_sb = const.tile([P, HO], f32)
            nc.scalar.dma_start(b2_sb[:], b2[:])
            w2_sb = const.tile([P, FO, H], dt_in)
            nper = FO // NSLAB2
            for q in range(NSLAB2):
                nc.scalar.dma_start(
                    w2_sb[:, q * nper : (q + 1) * nper],
                    w2[:, q * nper : (q + 1) * nper],
                )

            hT = const.tile([P, FO, C], dt_in)

            # phase 1: hT = gelu(w1^T-stationary matmul + b1)
            # chunk-innermost so consecutive matmuls share the stationary
            # operand (one LDWEIGHTS per (fo, ko) block).
            for fo in range(FO):
                s, q = divmod(fo, fper)
                pss = [
                    ps1.tile([P, 512], f32, tag="ps1", name="ps1t") for _ in chunks
                ]
                for ko in range(KO):
                    for ci, (c0, cw) in enumerate(chunks):
                        nc.tensor.matmul(
                            pss[ci][:, :cw],
                            w1_sb[:, s, ko, q * P : (q + 1) * P],
                            xT_sb[:, KO * c0 + ko * cw : KO * c0 + (ko + 1) * cw],
                            start=(ko == 0),
                            stop=(ko == KO - 1),
                        )
                for ci, (c0, cw) in enumerate(chunks):
                    nc.scalar.activation(
                        hT[:, fo, c0 : c0 + cw],
                        pss[ci][:, :cw],
                        mybir.ActivationFunctionType.Gelu,
                        bias=b1_sb[:, fo : fo + 1],
                    )

            # phase 2: yT = w2^T-stationary matmul over hT + b2
            for ho in range(HO):
                pss = [
                    ps2.tile([P, 512], f32, tag="ps2", name="ps2t") for _ in chunks
                ]
                for fo in range(FO):
                    for ci, (c0, cw) in enumerate(chunks):
                        nc.tensor.matmul(
                            pss[ci][:, :cw],
                            w2_sb[:, fo, ho * P : (ho + 1) * P],
                            hT[:, fo, c0 : c0 + cw],
                            start=(fo == 0),
                            stop=(fo == FO - 1),
                        )
                for ci, (c0, cw) in enumerate(chunks):
                    ysb = yp.tile([P, 512], f32, tag="ysb")
                    nc.scalar.activation(
                        ysb[:, :cw],
                        pss[ci][:, :cw],
                        mybir.ActivationFunctionType.Identity,
                        bias=b2_sb[:, ho : ho + 1],
                    )
                    nc.sync.dma_start(yT[:, ho, c0 : c0 + cw], ysb[:, :cw])
    nc.compile()
    return nc


def _get_nc(C, dt_tag):
    key = (C, dt_tag)
    if key not in _nc_cache:
        _nc_cache[key] = _build(C, dt_tag)
    return _nc_cache[key]


def _route(x, router_w):
    """Top-2 routing identical (up to fp noise far below the tie margin)
    to jax.lax.top_k + softmax in the reference."""
    n = x.shape[0]
    logits = x.astype(np.float64) @ router_w.astype(np.float64)
    r = np.arange(n)
    i1 = np.argmax(logits, 1)
    masked = logits.copy()
    masked[r, i1] = -np.inf
    i2 = np.argmax(masked, 1)
    tl = np.stack([logits[r, i1], logits[r, i2]], 1).astype(np.float32)
    e = np.exp(tl - tl.max(1, keepdims=True))
    s = (e / e.sum(1, keepdims=True)).astype(np.float32)
    return np.stack([i1, i2], 1), s


def _np_dtype(dt_tag):
    if dt_tag == "bf16":
        import ml_dtypes

        return ml_dtypes.bfloat16
    if dt_tag == "fp16":
        return np.float16
    return np.float32


def _prepare(inputs, dt_tag):
    hs = np.asarray(inputs["hidden_states"], np.float32)
    router_w = np.asarray(inputs["router_w"], np.float32)
    w1 = np.asarray(inputs["w1"], np.float32)
    b1 = np.asarray(inputs["b1"], np.float32)
    w2 = np.asarray(inputs["w2"], np.float32)
    b2 = np.asarray(inputs["b2"], np.float32)
    S, B, H_ = hs.shape
    x = hs.reshape(S * B, H_)

    idx2, scores = _route(x, router_w)
    tok = [np.flatnonzero((idx2 == e).any(1)) for e in range(E)]
    wts = []
    for e in range(E):
        sel = idx2[tok[e]] == e  # [n_e, 2]; exactly one True per row
        wts.append(
            np.where(sel[:, 0], scores[tok[e], 0], scores[tok[e], 1]).astype(
                np.float32
            )
        )

    maxn = max(len(t) for t in tok)
    C = max(64, ((maxn + 7) // 8) * 8)
    chunks = _chunks(C)

    np_in = _np_dtype(dt_tag)
    fcols = F // NSLAB1

    in_maps = []
    for e in range(E):
        n_e = len(tok[e])
        xT3 = np.zeros((P, KO, C), np_in)
        xT3[:, :, :n_e] = x[tok[e]].T.reshape(KO, P, n_e).transpose(1, 0, 2)
        # chunk-major flat layout: each chunk's KO k-rows contiguous
        xT = np.concatenate(
            [xT3[:, :, c0 : c0 + cw].reshape(P, KO * cw) for c0, cw in chunks],
            axis=1,
        )
        in_maps.append(
            {
                "xT": np.ascontiguousarray(xT),
                "w1": np.ascontiguousarray(
                    w1[e]
                    .reshape(KO, P, NSLAB1, fcols)
                    .transpose(1, 2, 0, 3)
                    .astype(np_in)
                ),
                "b1": np.ascontiguousarray(b1[e].reshape(FO, P).T),
                "w2": np.ascontiguousarray(
                    w2[e].reshape(FO, P, H_).transpose(1, 0, 2).astype(np_in)
                ),
                "b2": np.ascontiguousarray(b2[e].reshape(HO, P).T),
            }
        )
    return (S, B, H_), x, tok, wts, C, in_maps


def _combine(shape, tok, wts, results):
    S, B, H_ = shape
    out = np.zeros((S * B, H_), np.float32)
    for e in range(E):
        n_e = len(tok[e])
        yT = results[e]["yT"]  # [P, HO, C] f32
        y = yT.transpose(1, 0, 2).reshape(H_, -1)[:, :n_e].T
        out[tok[e]] += wts[e][:, None] * y
    return out.reshape(S, B, H_)


def kernel(**inputs):
    dt_tag = KERNEL_DTYPE
    shape, _x, tok, wts, C, in_maps = _prepare(inputs, dt_tag)
    nc = _get_nc(C, dt_tag)
    res = run_bass_kernel_spmd(nc, in_maps, core_ids=list(range(NCORES)))
    return _combine(shape, tok, wts, res.results)


# revision 8
# speedup vs baseline: 1.2573x; 1.2573x over previous
"""MoE routed-MLP (GPTNeoX) Trainium2 kernel.

Expert-parallel over 8 NeuronCores: core e holds expert e's weights.
Host computes the (tiny) router + top-2 dispatch, gathers each expert's
tokens into a padded batch, and scatter-adds the weighted expert outputs
back. Each core runs the same SPMD Bass program:

    hT[f, c] = gelu( sum_k w1[k, f] * xT[k, c] + b1[f] )      (f on partitions)
    yT[h, c] = sum_f w2[f, h] * hT[f, c] + b2[h]              (h on partitions)

Both matmuls keep the weights as the stationary operand so the
intermediate never needs an on-chip transpose. All weights are resident
in SBUF at 16-bit. Every DMA source is contiguous per partition; the
trigger instructions are split across the Sync (w1, y) and Activation
(x, biases, w2) sequencers so the first matmul's inputs land as early as
possible, and a short chain of warm-up matmuls on a memset tile ramps
the PE to its top p-state while those DMAs drain.
"""

import numpy as np

import concourse.bass as bass  # noqa: F401  (bass types used via tile/bacc)
import concourse.mybir as mybir
import concourse.tile as tile
from concourse import bacc
from concourse.bass_utils import run_bass_kernel_spmd

H = 1024
F = 4096
E = 8
NCORES = 8
P = 128
KO = H // P  # 8   k-chunks for the H contraction
FO = F // P  # 32  f-tiles
HO = H // P  # 8   h-tiles
# w1 DMA slab widths in F-columns: small first slabs so the opening
# matmuls' weights arrive ASAP, big ones later (fewer triggers).
W1_SLABS = [256, 256, 512, 512, 512, 512, 512, 512, 512]
NSLAB2 = 4  # w2 DMA slabs (8 f-tiles each)
N_WARMUP = 26  # PE warm-up matmuls (~3.3us to reach top p-state)

# "fp16" | "bf16" | "fp32r" | "fp32"
KERNEL_DTYPE = "fp16"

_nc_cache = {}


def _chunks(C):
    """Split [0, C) into column chunks, each <= 512 (one PSUM bank of f32),
    as balanced as possible in multiples of 8."""
    n = (C + 511) // 512
    base = (C // n) // 8 * 8
    widths = [base] * n
    rem = C - base * n
    i = 0
    while rem > 0:
        widths[i] += 8
        rem -= 8
        i = (i + 1) % n
    out, off = [], 0
    for w in widths:
        out.append((off, w))
        off += w
    return out


def _w1_slab_of(fo):
    """(slab f0, slab width) containing f-tile fo."""
    f0 = 0
    for fw in W1_SLABS:
        if fo * P < f0 + fw:
            return f0, fw
        f0 += fw
    raise AssertionError(fo)


def _build(C, dt_tag):
    f32 = mybir.dt.float32
    dt_in = {
        "fp16": mybir.dt.float16,
        "bf16": mybir.dt.bfloat16,
        "fp32r": mybir.dt.float32r,
        "fp32": f32,
    }[dt_tag]
    assert sum(W1_SLABS) == F

    nc = bacc.Bacc("TRN2", target_bir_lowering=False, debug=False)
    # x is chunk-major: for each column chunk, its KO k-rows are contiguous.
    xT = nc.dram_tensor("xT", [P, KO * C], dt_in, kind="ExternalInput").ap()
    # w1 is slab-major: slab s (fw F-cols) occupies KO*fw contiguous cols.
    w1 = nc.dram_tensor("w1", [P, KO * F], dt_in, kind="ExternalInput").ap()
    b1 = nc.dram_tensor("b1", [P, FO], f32, kind="ExternalInput").ap()
    w2 = nc.dram_tensor("w2", [P, FO, H], dt_in, kind="ExternalInput").ap()
    b2 = nc.dram_tensor("b2", [P, HO], f32, kind="ExternalInput").ap()
    yT = nc.dram_tensor("yT", [P, HO, C], f32, kind="ExternalOutput").ap()
    chunks = _chunks(C)

    with tile.TileContext(nc) as tc:
        with (
            tc.tile_pool(name="const", bufs=1) as const,
            tc.tile_pool(name="yp", bufs=4) as yp,
            tc.tile_pool(name="psw", bufs=1, space="PSUM") as psw,
            tc.tile_pool(name="ps1", bufs=3, space="PSUM") as ps1,
            tc.tile_pool(name="ps2", bufs=4, space="PSUM") as ps2,
        ):
            # --- PE warm-up: ramp to top p-state while input DMAs drain ---
            wu = const.tile([P, P], dt_in)
            nc.vector.memset(wu[:], 0.0)
            ps_wu = psw.tile([P, P], f32, tag="wu")
            for _ in range(N_WARMUP):
                nc.tensor.matmul(ps_wu[:], wu[:], wu[:], start=True, stop=True)

            # --- input DMAs: w1 slabs on Sync; x/biases/w2 on Scalar so the
            # two trigger streams run in parallel ---
            w1_sb = const.tile([P, KO * F], dt_in)
            f0 = 0
            for fw in W1_SLABS:
                nc.sync.dma_start(
                    w1_sb[:, KO * f0 : KO * (f0 + fw)],
                    w1[:, KO * f0 : KO * (f0 + fw)],
                )
                f0 += fw

            xT_sb = const.tile([P, KO * C], dt_in)
            (c00, cw0) = chunks[0]
            nc.scalar.dma_start(xT_sb[:, : KO * cw0], xT[:, : KO * cw0])
            b1_sb = const.tile([P, FO], f32)
            nc.scalar.dma_start(b1_sb[:], b1[:])
            if len(chunks) > 1:
                nc.scalar.dma_start(xT_sb[:, KO * cw0 :], xT[:, KO * cw0 :])
            b2_sb = const.tile([P, HO], f32)
            nc.scalar.dma_start(b2_sb[:], b2[:])
            w2_sb = const.tile([P, FO, H], dt_in)
            nper = FO // NSLAB2
            for q in range(NSLAB2):
                nc.scalar.dma_start(
                    w2_sb[:, q * nper : (q + 1) * nper],
                    w2[:, q * nper : (q + 1) * nper],
                )

            hT = const.tile([P, FO, C], dt_in)

            # phase 1: hT = gelu(w1^T-stationary matmul + b1)
            for fo in range(FO):
                f0, fw = _w1_slab_of(fo)
                base = KO * f0
                loc = fo * P - f0
                for c0, cw in chunks:
                    ps = ps1.tile([P, 512], f32, tag="ps1")
                    for ko in range(KO):
                        nc.tensor.matmul(
                            ps[:, :cw],
                            w1_sb[:, base + ko * fw + loc : base + ko * fw + loc + P],
                            xT_sb[:, KO * c0 + ko * cw : KO * c0 + (ko + 1) * cw],
                            start=(ko == 0),
                            stop=(ko == KO - 1),
                        )
                    nc.scalar.activation(
                        hT[:, fo, c0 : c0 + cw],
                        ps[:, :cw],
                        mybir.ActivationFunctionType.Gelu,
                        bias=b1_sb[:, fo : fo + 1],
                    )

            # phase 2: yT = w2^T-stationary matmul over hT + b2
            for ho in range(HO):
                for c0, cw in chunks:
                    ps = ps2.tile([P, 512], f32, tag="ps2")
                    for fo in range(FO):
                        nc.tensor.matmul(
                            ps[:, :cw],
                            w2_sb[:, fo, ho * P : (ho + 1) * P],
                            hT[:, fo, c0 : c0 + cw],
                            start=(fo == 0),
                            stop=(fo == FO - 1),
                        )
                    ysb = yp.tile([P, 512], f32, tag="ysb")
                    nc.scalar.activation(
                        ysb[:, :cw],
                        ps[:, :cw],
                        mybir.ActivationFunctionType.Identity,
                        bias=b2_sb[:, ho : ho + 1],
                    )
                    nc.sync.dma_start(yT[:, ho, c0 : c0 + cw], ysb[:, :cw])
    nc.compile()
    return nc


def _get_nc(C, dt_tag):
    key = (C, dt_tag)
    if key not in _nc_cache:
        _nc_cache[key] = _build(C, dt_tag)
    return _nc_cache[key]


def _route(x, router_w):
    """Top-2 routing identical (up to fp noise far below the tie margin)
    to jax.lax.top_k + softmax in the reference."""
    n = x.shape[0]
    logits = x.astype(np.float64) @ router_w.astype(np.float64)
    r = np.arange(n)
    i1 = np.argmax(logits, 1)
    masked = logits.copy()
    masked[r, i1] = -np.inf
    i2 = np.argmax(masked, 1)
    tl = np.stack([logits[r, i1], logits[r, i2]], 1).astype(np.float32)
    e = np.exp(tl - tl.max(1, keepdims=True))
    s = (e / e.sum(1, keepdims=True)).astype(np.float32)
    return np.stack([i1, i2], 1), s


def _np_dtype(dt_tag):
    if dt_tag == "bf16":
        import ml_dtypes

        return ml_dtypes.bfloat16
    if dt_tag == "fp16":
        return np.float16
    return np.float32


def _prepare(inputs, dt_tag):
    hs = np.asarray(inputs["hidden_states"], np.float32)
    router_w = np.asarray(inputs["router_w"], np.float32)
    w1 = np.asarray(inputs["w1"], np.float32)
    b1 = np.asarray(inputs["b1"], np.float32)
    w2 = np.asarray(inputs["w2"], np.float32)
    b2 = np.asarray(inputs["b2"], np.float32)
    S, B, H_ = hs.shape
    x = hs.reshape(S * B, H_)

    idx2, scores = _route(x, router_w)
    tok = [np.flatnonzero((idx2 == e).any(1)) for e in range(E)]
    wts = []
    for e in range(E):
        sel = idx2[tok[e]] == e  # [n_e, 2]; exactly one True per row
        wts.append(
            np.where(sel[:, 0], scores[tok[e], 0], scores[tok[e], 1]).astype(
                np.float32
            )
        )

    maxn = max(len(t) for t in tok)
    C = max(64, ((maxn + 7) // 8) * 8)
    chunks = _chunks(C)

    np_in = _np_dtype(dt_tag)

    in_maps = []
    for e in range(E):
        n_e = len(tok[e])
        xT3 = np.zeros((P, KO, C), np_in)
        xT3[:, :, :n_e] = x[tok[e]].T.reshape(KO, P, n_e).transpose(1, 0, 2)
        # chunk-major flat layout: each chunk's KO k-rows contiguous
        xT = np.concatenate(
            [xT3[:, :, c0 : c0 + cw].reshape(P, KO * cw) for c0, cw in chunks],
            axis=1,
        )
        # slab-major flat w1: slab s = [P, KO, fw] flattened
        w1e = w1[e].reshape(KO, P, F).astype(np_in)
        slabs = []
        f0 = 0
        for fw in W1_SLABS:
            slabs.append(
                w1e[:, :, f0 : f0 + fw].transpose(1, 0, 2).reshape(P, KO * fw)
            )
            f0 += fw
        in_maps.append(
            {
                "xT": np.ascontiguousarray(xT),
                "w1": np.ascontiguousarray(np.concatenate(slabs, axis=1)),
                "b1": np.ascontiguousarray(b1[e].reshape(FO, P).T),
                "w2": np.ascontiguousarray(
                    w2[e].reshape(FO, P, H_).transpose(1, 0, 2).astype(np_in)
                ),
                "b2": np.ascontiguousarray(b2[e].reshape(HO, P).T),
            }
        )
    return (S, B, H_), x, tok, wts, C, in_maps


def _combine(shape, tok, wts, results):
    S, B, H_ = shape
    out = np.zeros((S * B, H_), np.float32)
    for e in range(E):
        n_e = len(tok[e])
        yT = results[e]["yT"]  # [P, HO, C] f32
        y = yT.transpose(1, 0, 2).reshape(H_, -1)[:, :n_e].T
        out[tok[e]] += wts[e][:, None] * y
    return out.reshape(S, B, H_)


def kernel(**inputs):
    dt_tag = KERNEL_DTYPE
    shape, _x, tok, wts, C, in_maps = _prepare(inputs, dt_tag)
    nc = _get_nc(C, dt_tag)
    res = run_bass_kernel_spmd(nc, in_maps, core_ids=list(range(NCORES)))
    return _combine(shape, tok, wts, res.results)
